# revision 1
# baseline (speedup 1.0000x reference)
"""Trainium2 Bass kernel for nn_ExchangeBlock (GNN message passing / e3nn-style
tensor-product edge block), SPMD across 8 NeuronCores.

Sharding: edges across the 8 cores; node features and params replicated.

v2 design notes:
- All row gathers use the 16-lane GPSIMD dma_gather ucode (512-1024 indices
  per instruction) instead of per-row indirect DMA descriptor generation.
  dma_gather takes int16 indices, so node tables are stored as paired rows
  (25000 x 2*rowlen), indexed by node_id>>1 with an on-chip parity select.
- Two activation-table phases: phase A (exp table) computes geometry + RBF
  for all blocks (sqrt via Newton rsqrt on the VectorEngine); phase B (silu
  table) does everything else; Sin (envelope cosine) lives in the silu set.
- The tensor product runs as outer-product features P[e,1344] built on DVE
  with broadcast access patterns (bf16 for the big 0e x 0e block), PE
  transposes of P chunks (bf16: single-pass, fp32 would split into 2 ops),
  and accumulated 128x128 matmuls against pre-scaled flattened weights.
- LayerNorm affine is folded into a widened dfilter matmul; biases are added
  on DVE straight into PSUM (avoids rank-1 bias matmuls on the PE).
"""

import sys

sys.path.insert(0, "/opt/trn_rl_repo")

import numpy as np
import ml_dtypes

import concourse.bass as bass
import concourse.mybir as mybir
import concourse.tile as tile
from concourse import bacc
from concourse.bass_utils import run_bass_kernel_spmd
from concourse.masks import make_identity

F32 = mybir.dt.float32
BF16 = mybir.dt.bfloat16
I32 = mybir.dt.int32
I16 = mybir.dt.int16
AF = mybir.ActivationFunctionType
OP = mybir.AluOpType

# Problem constants
L0, L1, L2 = 32, 16, 8
NS = 128
NB = 64
CUTOFF = 7.0
N_NODES = 50000
N_EDGES = 400000
NODE_DIM = 120
NCORES = 8

BLK = 512             # edges per block
SUB = 4               # 128-edge sub-tiles per block
P = 128
KTP = 1344            # 1024 + 256 + 64 contraction size
KPAD = 1408           # padded to 11 chunks of 128
NCHUNK = 11
RSQRT_MAGIC = 0x5F3759DF
NPAIR = N_NODES // 2  # 25000
XROW = 128            # padded node row (124 used)
PROW = 32             # padded pos row (4 used)

E_CORE = N_EDGES // NCORES                      # 50000
NBLOCKS = (E_CORE + BLK - 1) // BLK             # 98
E_PAD = NBLOCKS * BLK                           # 50176

_compiled = None


def _patch_walrus_dge_levels():
    """This walrus build compiles with DynamicDMA disabled by default, which
    makes dynamic-offset DMAs crash the exec unit. Append the full
    --dge-levels set to every walrus invocation."""
    import concourse.bass_utils as _bu

    if getattr(_bu, "_dge_patched", False):
        return
    orig = _bu.run_command

    def patched(argv, **kw):
        if argv and "walrus_driver" in str(argv[0]) and not any(
            "dge-levels" in str(a) for a in argv
        ):
            argv = list(argv) + [
                "--dge-levels=io,spill_reload,scalar_dynamic_offset,"
                "vector_dynamic_offsets,dynamic_size,dst_reduce,transpose"
            ]
        return orig(argv, **kw)

    _bu.run_command = patched
    _bu._dge_patched = True


_patch_walrus_dge_levels()


def _patch_drain_and_barrier():
    """The final Tile drain runs on the SP engine, whose Drain lowering in this
    walrus build has no free sync-wait slots (its HWDGE queue waits fill them).
    Hoist the tile-clock waits onto dedicated nop instructions emitted just
    before the drain, one wait per nop."""
    if getattr(tile.TileContext, "_dab_patched", False):
        return

    def patched(self, tick_clock, wait_clock):
        nc = self.nc
        nops = [nc.sync.nop() for _ in range(32)]
        drain_inst = nc.sync.drain()
        from concourse.tile import ScopedClock

        wait_clock.add_sem_waits(
            drain_inst.ins, ScopedClock({None: tick_clock.global_clock})
        )
        si = drain_inst.ins.sync_info
        waits = list(si.on_wait) if si and si.on_wait else []
        if waits:
            assert len(waits) <= len(nops), f"{len(waits)} waits > nop slots"
            si.on_wait = []
            for w, n in zip(waits, nops):
                n.ins.sync_info = mybir.SyncInfo(on_wait=[w], on_update=[])

        nc.all_engine_barrier()
        assert self.sems is not None
        popped = nc._tile_sem_poison_stack.pop()
        assert popped is self._sem_poison
        nc.clear_and_free_semaphores(list(self.sems.allocated().values()))
        nc.all_engine_barrier()

    tile.TileContext._drain_and_barrier = patched
    tile.TileContext._dab_patched = True


_patch_drain_and_barrier()


def _newton_rsqrt(nc, pool, u, n, magic_t, tag):
    """rsqrt(u) for u[:, :n] > 0 on the VectorEngine (no ScalarE table)."""
    bits = pool.tile([P, n], I32, tag=f"{tag}_b")
    nc.vector.tensor_copy(out=bits[:].bitcast(F32), in_=u)  # raw bit copy
    nc.vector.tensor_scalar(
        out=bits[:], in0=bits[:], scalar1=1, scalar2=None,
        op0=OP.arith_shift_right,
    )
    yb = pool.tile([P, n], I32, tag=f"{tag}_y")
    nc.vector.tensor_tensor(
        out=yb[:], in0=magic_t[:, 0:1].to_broadcast([P, n]), in1=bits[:],
        op=OP.subtract,
    )
    y = yb[:].bitcast(F32)
    t1 = pool.tile([P, n], F32, tag=f"{tag}_t1")
    for _ in range(3):
        nc.vector.tensor_mul(t1[:], y, y)
        nc.vector.tensor_mul(t1[:], t1[:], u)
        nc.vector.tensor_scalar(
            out=t1[:], in0=t1[:], scalar1=-0.5, scalar2=1.5, op0=OP.mult, op1=OP.add,
        )
        nc.vector.tensor_mul(y, y, t1[:])
    return yb


def _build(nblocks: int):
    import os
    stage = os.environ.get("K_STAGE", "full")
    nc = bacc.Bacc("TRN2", target_bir_lowering=False, debug=False)

    nodes_pair = nc.dram_tensor("nodes_pair", (NPAIR, 2 * XROW), F32, kind="ExternalInput").ap()
    pos_pair = nc.dram_tensor("pos_pair", (NPAIR, 2 * PROW), F32, kind="ExternalInput").ap()
    cell64 = nc.dram_tensor("cell64", (32, 64), F32, kind="ExternalInput").ap()
    xw16 = nc.dram_tensor("xw16", (nblocks, P, 64), I16, kind="ExternalInput").ap()
    gbw16 = nc.dram_tensor("gbw16", (nblocks, P, 32), I16, kind="ExternalInput").ap()
    par = nc.dram_tensor("par", (nblocks * BLK, 2), F32, kind="ExternalInput").ap()
    eshift = nc.dram_tensor("eshift", (nblocks * BLK, 3), F32, kind="ExternalInput").ap()
    wflat = nc.dram_tensor("wflat", (KPAD, NS), BF16, kind="ExternalInput").ap()
    dfw1 = nc.dram_tensor("dfw1", (NB, 128), BF16, kind="ExternalInput").ap()
    dfb1 = nc.dram_tensor("dfb1", (1, 128), F32, kind="ExternalInput").ap()
    dfw2gb = nc.dram_tensor("dfw2gb", (128, 256), BF16, kind="ExternalInput").ap()
    dfb2gb = nc.dram_tensor("dfb2gb", (1, 256), F32, kind="ExternalInput").ap()
    mlpw1 = nc.dram_tensor("mlpw1", (128, 512), BF16, kind="ExternalInput").ap()
    mlpb1 = nc.dram_tensor("mlpb1", (1, 512), F32, kind="ExternalInput").ap()
    w2row = nc.dram_tensor("w2row", (1, 512), BF16, kind="ExternalInput").ap()
    b2sc = nc.dram_tensor("b2sc", (1, 1), F32, kind="ExternalInput").ap()
    offs = nc.dram_tensor("offs", (1, NB), F32, kind="ExternalInput").ap()
    out = nc.dram_tensor("out", (nblocks * BLK,), F32, kind="ExternalOutput").ap()

    width = CUTOFF / (NB - 1)
    coeff = 0.5 / (width * width)
    sqc = float(np.sqrt(coeff))

    with tile.TileContext(nc) as tc:
        with (
            tc.tile_pool(name="const", bufs=1) as constp,
            tc.tile_pool(name="io", bufs=3) as iop,
            tc.tile_pool(name="geo", bufs=3) as geop,
            tc.tile_pool(name="pfeat", bufs=2) as pfp,
            tc.tile_pool(name="trsb", bufs=3) as trsbp,
            tc.tile_pool(name="work", bufs=3) as workp,
            tc.tile_pool(name="gbig", bufs=2) as gbigp,
            tc.tile_pool(name="acc", bufs=2) as accp,
            tc.tile_pool(name="ps_tr", bufs=2, space="PSUM") as ps_tr,
            tc.tile_pool(name="ps_mm", bufs=2, space="PSUM") as ps_mm,
            tc.tile_pool(name="ps_h", bufs=1, space="PSUM") as ps_h,
            tc.tile_pool(name="ps_df", bufs=1, space="PSUM") as ps_df,
            tc.tile_pool(name="ps_g", bufs=2, space="PSUM") as ps_g,
        ):
            # ---- resident constants ----
            identb = constp.tile([P, P], BF16)
            make_identity(nc, identb[:])
            eps_t = constp.tile([P, 1], F32)
            nc.vector.memset(eps_t[:], 1e-5)
            nhalfpi_t = constp.tile([P, 1], F32)
            nc.vector.memset(nhalfpi_t[:], float(-np.pi / 2))
            magic_t = constp.tile([P, 1], I32)
            nc.vector.memset(magic_t[:], RSQRT_MAGIC)

            w_sb = constp.tile([P, NCHUNK, P], BF16)
            nc.sync.dma_start(out=w_sb[:], in_=wflat.rearrange("(c p) w -> p c w", p=P))
            dfw1_sb = constp.tile([NB, 128], BF16)
            nc.sync.dma_start(out=dfw1_sb[:], in_=dfw1)
            dfw2gb_sb = constp.tile([128, 256], BF16)
            nc.sync.dma_start(out=dfw2gb_sb[:], in_=dfw2gb)
            mlpw1_sb = constp.tile([128, 512], BF16)
            nc.sync.dma_start(out=mlpw1_sb[:], in_=mlpw1)
            dfb1_rep = constp.tile([P, 128], F32)
            nc.sync.dma_start(out=dfb1_rep[:], in_=dfb1.to_broadcast([P, 128]))
            dfb2gb_rep = constp.tile([P, 256], F32)
            nc.sync.dma_start(out=dfb2gb_rep[:], in_=dfb2gb.to_broadcast([P, 256]))
            mlpb1_rep = constp.tile([P, 512], F32)
            nc.sync.dma_start(out=mlpb1_rep[:], in_=mlpb1.to_broadcast([P, 512]))
            w2rep_sb = constp.tile([P, 512], BF16)
            nc.sync.dma_start(out=w2rep_sb[:], in_=w2row.to_broadcast([P, 512]))
            b2_sb = constp.tile([P, 1], F32)
            nc.sync.dma_start(out=b2_sb[:], in_=b2sc.to_broadcast([P, 1]))
            offs_sb = constp.tile([P, NB], F32)
            nc.sync.dma_start(out=offs_sb[:], in_=offs.to_broadcast([P, NB]))

            # phase A -> phase B hand-off (resident)
            rbf_store = constp.tile([P, nblocks, SUB, NB], BF16)
            dist_store = constp.tile([P, nblocks, SUB], F32)

            # =========== Phase A: geometry + RBF (exp table) ===========
            for b in range(nblocks):
                e0 = b * BLK
                sl = slice(e0, e0 + BLK)
                xw = iop.tile([P, 64], I16, tag="xw")
                nc.sync.dma_start(out=xw[:], in_=xw16[b])
                gw = iop.tile([P, 32], I16, tag="gw")
                nc.sync.dma_start(out=gw[:], in_=gbw16[b])
                prt = iop.tile([P, SUB, 2], F32, tag="prt")
                nc.sync.dma_start(out=prt[:], in_=par[sl, :].rearrange("(s p) j -> p s j", p=P))
                esh = iop.tile([P, SUB, 3], F32, tag="esh")
                nc.sync.dma_start(out=esh[:], in_=eshift[sl, :].rearrange("(s p) j -> p s j", p=P))

                pg = geop.tile([P, 2 * SUB, 2 * PROW], F32, tag="pg")
                nc.gpsimd.dma_gather(
                    out_ap=pg[:], in_ap=pos_pair[:, :], idxs_ap=xw[:],
                    num_idxs=2 * BLK, num_idxs_reg=2 * BLK, elem_size=2 * PROW,
                )
                bcg = geop.tile([P, SUB, 64], F32, tag="bcg")
                nc.gpsimd.dma_gather(
                    out_ap=bcg[:], in_ap=cell64[:, :], idxs_ap=gw[:],
                    num_idxs=BLK, num_idxs_reg=BLK, elem_size=64,
                )

                # parity select: pos_i[p,s,0:4] = even/odd row half by parity
                pos1 = geop.tile([P, SUB, 4], F32, tag="pos1")
                pos2 = geop.tile([P, SUB, 4], F32, tag="pos2")
                posh = geop.tile([P, 2, SUB, 4], F32, tag="posh")
                pmsk = geop.tile([P, 2, SUB, 4], mybir.dt.uint8, tag="pmsk")
                nc.gpsimd.tensor_copy(out=pos1[:], in_=pg[:, 0:SUB, 0:4])
                nc.gpsimd.tensor_copy(out=pos2[:], in_=pg[:, SUB : 2 * SUB, 0:4])
                nc.gpsimd.tensor_copy(
                    out=posh[:].rearrange("p e s j -> p (e s) j"),
                    in_=pg[:, :, PROW : PROW + 4],
                )
                nc.gpsimd.tensor_copy(
                    out=pmsk[:],
                    in_=prt[:].transpose([0, 2, 1]).unsqueeze(3).to_broadcast([P, 2, SUB, 4]),
                )
                nc.vector.copy_predicated(
                    out=pos1[:].rearrange("p s j -> p (s j)"),
                    mask=pmsk[:, 0, :, :].rearrange("p s j -> p (s j)"),
                    data=posh[:, 0, :, :].rearrange("p s j -> p (s j)"),
                )
                nc.vector.copy_predicated(
                    out=pos2[:].rearrange("p s j -> p (s j)"),
                    mask=pmsk[:, 1, :, :].rearrange("p s j -> p (s j)"),
                    data=posh[:, 1, :, :].rearrange("p s j -> p (s j)"),
                )

                # tvec[p,s,j] = sum_i esh[p,s,i] * bc[p,s,3i+j]
                tvp = geop.tile([P, SUB, 3, 3], F32, tag="tvp")
                nc.vector.tensor_tensor(
                    out=tvp[:],
                    in0=esh[:].unsqueeze(3).to_broadcast([P, SUB, 3, 3]),
                    in1=bcg[:, :, 0:9].rearrange("p s (i j) -> p s i j", j=3),
                    op=OP.mult,
                )
                tv = geop.tile([P, SUB, 3], F32, tag="tv")
                nc.vector.reduce_sum(
                    out=tv[:], in_=tvp[:].transpose([0, 1, 3, 2]), axis=mybir.AxisListType.X,
                )
                rv = geop.tile([P, SUB, 3], F32, tag="rv")
                nc.vector.tensor_sub(rv[:], pos2[:, :, 0:3], pos1[:, :, 0:3])
                nc.vector.tensor_add(rv[:], rv[:], tv[:])
                rv2 = geop.tile([P, SUB, 3], F32, tag="rv2")
                nc.vector.tensor_mul(rv2[:], rv[:], rv[:])
                d2 = geop.tile([P, SUB], F32, tag="d2")
                nc.vector.reduce_sum(out=d2[:], in_=rv2[:], axis=mybir.AxisListType.X)
                nc.vector.tensor_scalar(
                    out=d2[:], in0=d2[:], scalar1=1e-12, scalar2=None, op0=OP.max,
                )
                ry = _newton_rsqrt(nc, geop, d2[:], SUB, magic_t, "rsq")
                dist = dist_store[:, b, :]
                nc.vector.tensor_mul(dist, d2[:], ry[:].bitcast(F32))

                rb = geop.tile([P, SUB, NB], F32, tag="rb")
                nc.vector.tensor_tensor(
                    out=rb[:],
                    in0=offs_sb[:].unsqueeze(1).to_broadcast([P, SUB, NB]),
                    in1=dist.unsqueeze(2).to_broadcast([P, SUB, NB]),
                    op=OP.subtract,
                )
                nc.scalar.activation(rb[:], rb[:], AF.Square, scale=sqc)
                nc.scalar.activation(rbf_store[:, b, :, :], rb[:], AF.Exp, scale=-1.0)

            if stage == "geo":
                for b in range(nblocks):
                    acc = accp.tile([P, SUB], F32, tag="acc")
                    nc.vector.tensor_copy(out=acc[:], in_=dist_store[:, b, :])
                    nc.sync.dma_start(
                        out=out[b * BLK : (b + 1) * BLK].rearrange("(s p) -> p s", p=P),
                        in_=acc[:],
                    )

            # =========== Phase B: gathers + TP + MLPs (silu table) ===========
            for b in range(nblocks if stage != "geo" else 0):
                e0 = b * BLK
                sl = slice(e0, e0 + BLK)
                xw = iop.tile([P, 64], I16, tag="xw")
                nc.sync.dma_start(out=xw[:], in_=xw16[b])
                prt = iop.tile([P, SUB, 2], F32, tag="prt")
                nc.sync.dma_start(out=prt[:], in_=par[sl, :].rearrange("(s p) j -> p s j", p=P))

                xg = gbigp.tile([P, 2 * SUB, 2 * XROW], F32, tag="xg")
                nc.gpsimd.dma_gather(
                    out_ap=xg[:], in_ap=nodes_pair[:, :], idxs_ap=xw[:],
                    num_idxs=2 * BLK, num_idxs_reg=2 * BLK, elem_size=2 * XROW,
                )
                x1 = gbigp.tile([P, SUB, 124], F32, tag="x1")
                x2 = gbigp.tile([P, SUB, 124], F32, tag="x2")
                xh = gbigp.tile([P, 2, SUB, 124], F32, tag="xh")
                xmsk = gbigp.tile([P, 2, SUB, 124], mybir.dt.uint8, tag="xmsk")
                nc.gpsimd.tensor_copy(out=x1[:], in_=xg[:, 0:SUB, 0:124])
                nc.gpsimd.tensor_copy(out=x2[:], in_=xg[:, SUB : 2 * SUB, 0:124])
                nc.gpsimd.tensor_copy(
                    out=xh[:].rearrange("p e s j -> p (e s) j"),
                    in_=xg[:, :, XROW : XROW + 124],
                )
                nc.gpsimd.tensor_copy(
                    out=xmsk[:],
                    in_=prt[:].transpose([0, 2, 1]).unsqueeze(3).to_broadcast([P, 2, SUB, 124]),
                )
                nc.vector.copy_predicated(
                    out=x1[:].rearrange("p s j -> p (s j)"),
                    mask=xmsk[:, 0, :, :].rearrange("p s j -> p (s j)"),
                    data=xh[:, 0, :, :].rearrange("p s j -> p (s j)"),
                )
                nc.vector.copy_predicated(
                    out=x2[:].rearrange("p s j -> p (s j)"),
                    mask=xmsk[:, 1, :, :].rearrange("p s j -> p (s j)"),
                    data=xh[:, 1, :, :].rearrange("p s j -> p (s j)"),
                )

                dist = dist_store[:, b, :]
                dc = geop.tile([P, SUB], F32, tag="dc")
                nc.vector.tensor_scalar(
                    out=dc[:], in0=dist, scalar1=CUTOFF, scalar2=None, op0=OP.min,
                )
                cosd = geop.tile([P, SUB], F32, tag="cosd")
                nc.scalar.activation(
                    cosd[:], dc[:], AF.Sin,
                    bias=nhalfpi_t[:, 0:1], scale=float(np.pi / CUTOFF),
                )
                mask = geop.tile([P, SUB], F32, tag="mask")
                nc.vector.tensor_scalar(
                    out=mask[:], in0=dist, scalar1=CUTOFF, scalar2=None, op0=OP.is_lt,
                )
                env = geop.tile([P, SUB], F32, tag="env")
                nc.vector.tensor_scalar(
                    out=env[:], in0=cosd[:], scalar1=-0.5, scalar2=0.5,
                    op0=OP.mult, op1=OP.add,
                )
                nc.vector.tensor_mul(env[:], env[:], mask[:])
                demb = geop.tile([P, SUB, NB], BF16, tag="demb")
                nc.vector.tensor_tensor(
                    out=demb[:], in0=rbf_store[:, b, :, :],
                    in1=env[:].unsqueeze(2).to_broadcast([P, SUB, NB]),
                    op=OP.mult,
                )

                if stage == "gather":
                    acc = accp.tile([P, SUB], F32, tag="acc")
                    nc.vector.reduce_sum(out=acc[:], in_=x1[:], axis=mybir.AxisListType.X)
                    nc.sync.dma_start(out=out[sl].rearrange("(s p) -> p s", p=P), in_=acc[:])
                    continue

                psmix = ps_mm.tile([P, SUB, NS], F32, tag="psmix")
                muv = geop.tile([P, SUB], F32, tag="muv")
                varv = geop.tile([P, SUB], F32, tag="varv")

                # ---- pass 1: tensor product per sub-tile ----
                for s in range(SUB):
                    ptb = pfp.tile([P, KPAD], BF16, tag="ptb")
                    nc.vector.memset(ptb[:, KTP:KPAD], 0.0)
                    a1 = x1[:, s, 0:L0]
                    a2 = x2[:, s, 0:L0]
                    nc.vector.tensor_tensor(
                        out=ptb[:, 0:1024].rearrange("p (u v) -> p u v", v=L0),
                        in0=a1.unsqueeze(2).to_broadcast([P, L0, L0]),
                        in1=a2.unsqueeze(1).to_broadcast([P, L0, L0]),
                        op=OP.mult,
                    )
                    b1 = x1[:, s, 32:80].rearrange("p (u m) -> p u m", m=3)
                    b2 = x2[:, s, 32:80].rearrange("p (u m) -> p u m", m=3)
                    pb = workp.tile([P, L1, L1, 3], F32, tag="pb")
                    nc.vector.tensor_tensor(
                        out=pb[:],
                        in0=b1.unsqueeze(2).to_broadcast([P, L1, L1, 3]),
                        in1=b2.unsqueeze(1).to_broadcast([P, L1, L1, 3]),
                        op=OP.mult,
                    )
                    pf = workp.tile([P, 320], F32, tag="pf")
                    nc.vector.reduce_sum(
                        out=pf[:, 0:256].rearrange("p (u v) -> p u v", v=L1),
                        in_=pb[:], axis=mybir.AxisListType.X,
                    )
                    c1 = x1[:, s, 80:120].rearrange("p (u m) -> p u m", m=5)
                    c2 = x2[:, s, 80:120].rearrange("p (u m) -> p u m", m=5)
                    pc = workp.tile([P, L2, L2, 5], F32, tag="pc")
                    nc.vector.tensor_tensor(
                        out=pc[:],
                        in0=c1.unsqueeze(2).to_broadcast([P, L2, L2, 5]),
                        in1=c2.unsqueeze(1).to_broadcast([P, L2, L2, 5]),
                        op=OP.mult,
                    )
                    nc.vector.reduce_sum(
                        out=pf[:, 256:320].rearrange("p (u v) -> p u v", v=L2),
                        in_=pc[:], axis=mybir.AxisListType.X,
                    )
                    nc.vector.tensor_copy(out=ptb[:, 1024:1344], in_=pf[:])

                    # transposes in groups of <=4 chunks -> one PSUM bank,
                    # one batched PSUM->SBUF copy per group
                    for g, chunks in enumerate(((0, 1, 2, 3), (4, 5, 6, 7), (8, 9, 10))):
                        ptp = ps_tr.tile([P, 4, P], BF16, tag="ptp")
                        for j, c in enumerate(chunks):
                            nc.tensor.transpose(
                                ptp[:, j, :], ptb[:, c * P : (c + 1) * P], identb[:]
                            )
                        pts = trsbp.tile([P, 4, P], BF16, tag="pts")
                        ncopy = len(chunks)
                        if g == 1:
                            nc.scalar.copy(pts[:, 0:ncopy, :], ptp[:, 0:ncopy, :])
                        else:
                            nc.vector.tensor_copy(pts[:, 0:ncopy, :], ptp[:, 0:ncopy, :])
                        for j, c in enumerate(chunks):
                            nc.tensor.matmul(
                                psmix[:, s, :], lhsT=pts[:, j, :], rhs=w_sb[:, c, :],
                                start=(c == 0), stop=(c == NCHUNK - 1),
                            )

                    stats = geop.tile([P, 6], F32, tag="stats")
                    nc.vector.bn_stats(out=stats[:], in_=psmix[:, s, :])
                    mv = geop.tile([P, 2], F32, tag="mv")
                    nc.vector.bn_aggr(out=mv[:], in_=stats[:])
                    nc.vector.tensor_copy(out=muv[:, s : s + 1], in_=mv[:, 0:1])
                    nc.vector.tensor_copy(out=varv[:, s : s + 1], in_=mv[:, 1:2])

                if stage == "tp":
                    acc = accp.tile([P, SUB], F32, tag="acc")
                    nc.vector.tensor_copy(out=acc[:], in_=muv[:])
                    nc.sync.dma_start(out=out[sl].rearrange("(s p) -> p s", p=P), in_=acc[:])
                    continue

                # ---- block-level LN rstd ----
                nc.vector.tensor_scalar(
                    out=varv[:], in0=varv[:], scalar1=1e-5, scalar2=None, op0=OP.add,
                )
                ryl = _newton_rsqrt(nc, geop, varv[:], SUB, magic_t, "lnr")
                rstd = ryl[:].bitcast(F32)
                tb = geop.tile([P, SUB], F32, tag="tb")
                nc.vector.tensor_mul(tb[:], muv[:], rstd)
                nc.vector.tensor_scalar(
                    out=tb[:], in0=tb[:], scalar1=-1.0, scalar2=None, op0=OP.mult,
                )

                acc = accp.tile([P, SUB], F32, tag="acc")

                # ---- pass 2: LN apply + dfilter + final MLP ----
                for s in range(SUB):
                    ynorm = workp.tile([P, NS], BF16, tag="ynorm")
                    nc.scalar.activation(
                        ynorm[:], psmix[:, s, :], AF.Identity,
                        bias=tb[:, s : s + 1], scale=rstd[:, s : s + 1],
                    )

                    dT_ps = ps_tr.tile([P, 4, P], BF16, tag="ptp")
                    nc.tensor.transpose(dT_ps[0:NB, 0, :], demb[:, s, :], identb[:])
                    dT = trsbp.tile([NB, P], BF16, tag="dT")
                    nc.scalar.copy(dT[:], dT_ps[0:NB, 0, :])
                    ph = ps_h.tile([P, 128], F32, tag="ph")
                    nc.tensor.matmul(ph[:], lhsT=dT[:], rhs=dfw1_sb[:], start=True, stop=True)
                    nc.vector.tensor_add(ph[:], ph[:], dfb1_rep[:])
                    sact = workp.tile([P, 128], BF16, tag="sact")
                    nc.scalar.activation(sact[:], ph[:], AF.Silu)
                    sT_ps = ps_tr.tile([P, 4, P], BF16, tag="ptp")
                    nc.tensor.transpose(sT_ps[:, 0, :], sact[:], identb[:])
                    sT = trsbp.tile([P, P], BF16, tag="sT")
                    nc.vector.tensor_copy(sT[:], sT_ps[:, 0, :])
                    pdf = ps_df.tile([P, 256], F32, tag="pdf")
                    nc.tensor.matmul(pdf[:], lhsT=sT[:], rhs=dfw2gb_sb[:], start=True, stop=True)
                    dfs = workp.tile([P, 256], BF16, tag="dfs")
                    nc.vector.tensor_add(dfs[:], pdf[:], dfb2gb_rep[:])

                    rg = workp.tile([P, 128], BF16, tag="rg")
                    nc.vector.tensor_mul(rg[:], ynorm[:], dfs[:, 0:128])
                    nc.vector.tensor_add(rg[:], rg[:], dfs[:, 128:256])

                    rT_ps = ps_tr.tile([P, 4, P], BF16, tag="ptp")
                    nc.tensor.transpose(rT_ps[:, 0, :], rg[:], identb[:])
                    rT = trsbp.tile([P, P], BF16, tag="rT")
                    nc.scalar.copy(rT[:], rT_ps[:, 0, :])
                    pg2 = ps_g.tile([P, 512], F32, tag="pg")
                    nc.tensor.matmul(pg2[:], lhsT=rT[:], rhs=mlpw1_sb[:], start=True, stop=True)
                    nc.vector.tensor_add(pg2[:], pg2[:], mlpb1_rep[:])
                    gact = gbigp.tile([P, 512], BF16, tag="gact")
                    nc.scalar.activation(gact[:], pg2[:], AF.Silu)
                    scr = gbigp.tile([P, 512], BF16, tag="scr")
                    nc.vector.tensor_mul(scr[:], gact[:], w2rep_sb[:])
                    nc.vector.reduce_sum(
                        out=acc[:, s : s + 1], in_=scr[:], axis=mybir.AxisListType.X,
                    )

                nc.vector.tensor_scalar(
                    out=acc[:], in0=acc[:], scalar1=b2_sb[:, 0:1], scalar2=None,
                    op0=OP.add,
                )
                nc.sync.dma_start(out=out[sl].rearrange("(s p) -> p s", p=P), in_=acc[:])

    nc.compile()
    return nc


def _get_compiled():
    global _compiled
    if _compiled is None:
        _compiled = _build(NBLOCKS)
    return _compiled


def _wrap16(idx_block):
    """int array [512] -> dma_gather wrapped int16 layout [128, 32]
    (index j at [j%16, j//16], replicated across the 8 gpsimd cores)."""
    w = idx_block.astype(np.int16).reshape(-1, 16).T  # [16, n/16]
    return np.tile(w, (8, 1))


def _prep(inputs):
    nodes = np.asarray(inputs["nodes"], np.float32)
    edge_index = np.asarray(inputs["edge_index"]).astype(np.int64)
    graph_batch = np.asarray(inputs["graph_batch"]).astype(np.int64)
    cell = np.asarray(inputs["cell"], np.float32)
    edge_shift = np.asarray(inputs["edge_shift"], np.float32)
    pos = np.asarray(inputs["pos"], np.float32)

    nodes_pad = np.zeros((N_NODES, XROW), np.float32)
    nodes_pad[:, :NODE_DIM] = nodes
    nodes_pad[:, 120:123] = pos
    nodes_pad[:, 123] = graph_batch
    nodes_pair = nodes_pad.reshape(NPAIR, 2 * XROW)

    pos_pad = np.zeros((N_NODES, PROW), np.float32)
    pos_pad[:, 0:3] = pos
    pos_pair = pos_pad.reshape(NPAIR, 2 * PROW)

    cell64 = np.zeros((32, 64), np.float32)
    cell64[:, 0:9] = cell.reshape(32, 9)

    alpha = 1.0 / np.sqrt(float(L0 * L0 + L1 * L1 + L2 * L2))
    w0 = np.asarray(inputs["W0"], np.float32).reshape(L0 * L0, NS) * alpha
    w1 = np.asarray(inputs["W1"], np.float32).reshape(L1 * L1, NS) * (alpha / np.sqrt(3.0))
    w2 = np.asarray(inputs["W2"], np.float32).reshape(L2 * L2, NS) * (alpha / np.sqrt(5.0))
    wflat = np.zeros((KPAD, NS), np.float32)
    wflat[0:1024] = w0
    wflat[1024:1280] = w1
    wflat[1280:1344] = w2

    ln_g = np.asarray(inputs["ln_g"], np.float32)
    ln_b = np.asarray(inputs["ln_b"], np.float32)
    df_w2 = np.asarray(inputs["df_w2"], np.float32)
    df_b2 = np.asarray(inputs["df_b2"], np.float32)
    dfw2gb = np.concatenate([df_w2 * ln_g[None, :], df_w2 * ln_b[None, :]], axis=1)
    dfb2gb = np.concatenate([df_b2 * ln_g, df_b2 * ln_b])[None, :]

    bf = lambda a: np.ascontiguousarray(a).astype(ml_dtypes.bfloat16)

    common = {
        "nodes_pair": nodes_pair,
        "pos_pair": pos_pair,
        "cell64": cell64,
        "wflat": bf(wflat),
        "dfw1": bf(np.asarray(inputs["df_w1"], np.float32)),
        "dfb1": np.asarray(inputs["df_b1"], np.float32)[None, :],
        "dfw2gb": bf(dfw2gb),
        "dfb2gb": np.ascontiguousarray(dfb2gb.astype(np.float32)),
        "mlpw1": bf(np.asarray(inputs["mlp_w1"], np.float32)),
        "mlpb1": np.asarray(inputs["mlp_b1"], np.float32)[None, :],
        "w2row": bf(np.asarray(inputs["mlp_w2"], np.float32).T),
        "b2sc": np.asarray(inputs["mlp_b2"], np.float32).reshape(1, 1),
        "offs": np.linspace(0.0, CUTOFF, NB, dtype=np.float32)[None, :],
    }

    nblocks = E_PAD // BLK
    in_maps = []
    for c in range(NCORES):
        lo, hi = c * E_CORE, (c + 1) * E_CORE
        src = np.zeros(E_PAD, np.int64)
        dst = np.zeros(E_PAD, np.int64)
        esh = np.zeros((E_PAD, 3), np.float32)
        src[: hi - lo] = edge_index[0, lo:hi]
        dst[: hi - lo] = edge_index[1, lo:hi]
        esh[: hi - lo] = edge_shift[lo:hi]

        xw = np.zeros((nblocks, P, 64), np.int16)
        gbw = np.zeros((nblocks, P, 32), np.int16)
        for b in range(nblocks):
            sb = src[b * BLK : (b + 1) * BLK]
            db = dst[b * BLK : (b + 1) * BLK]
            xw[b, :, 0:32] = _wrap16(sb >> 1)
            xw[b, :, 32:64] = _wrap16(db >> 1)
            gbw[b] = _wrap16(graph_batch[sb])
        parr = np.stack([(src & 1), (dst & 1)], axis=1).astype(np.float32)

        m = dict(common)
        m["xw16"] = xw
        m["gbw16"] = gbw
        m["par"] = parr
        m["eshift"] = esh
        in_maps.append(m)
    return in_maps


def kernel(**inputs) -> np.ndarray:
    nc = _get_compiled()
    in_maps = _prep(inputs)
    res = run_bass_kernel_spmd(nc, in_maps, core_ids=list(range(NCORES)))
    outs = [res.results[c]["out"][:E_CORE] for c in range(NCORES)]
    return np.concatenate(outs).reshape(N_EDGES, 1).astype(np.float32)



# revision 4
# speedup vs baseline: 1.5348x; 1.5348x over previous
"""Trainium2 Bass kernel for nn_ExchangeBlock (GNN message passing / e3nn-style
tensor-product edge block), SPMD across 8 NeuronCores.

Sharding: edges across the 8 cores; node features and params replicated.

v3 design notes:
- Edges are host-sorted into 4 parity classes (src&1, dst&1) so the pair-row
  parity select becomes a compile-time AP slice: no masks, no predicated
  copies, no gpsimd tensor_copy traffic at all.
- ONE dma_gather per block: the node table is bf16 pair rows (512B) with the
  fp32 position bit-packed into units 120:126 of each row, so geometry and
  the tensor product share the same gathered tile.  cell[graph_batch[src]]
  is a host-side index prep (like the baseline's graph_batch[src]) and is
  streamed per edge together with edge_shift.
- Blocks are processed in groups of 8 with two ScalarE activation-table
  phases per group (exp set: RBF; silu set: MLPs).  The cosine cutoff
  envelope is an exact degree-6 polynomial in d^2, so phase A needs no Sin.
- Bias adds ride the PE as rank-1 accumulate matmuls instead of DVE adds.
- The TP runs as outer-product features P[e,1344] built on DVE (bf16), PE
  transposes of P chunks, and accumulated 128x128 matmuls against
  pre-scaled flattened weights; the KPAD tail is junk x zero-W rows.
"""

import sys

sys.path.insert(0, "/opt/trn_rl_repo")

import numpy as np
import ml_dtypes

import concourse.bass as bass
import concourse.mybir as mybir
import concourse.tile as tile
from concourse import bacc
from concourse.bass_utils import run_bass_kernel_spmd
from concourse.masks import make_identity

F32 = mybir.dt.float32
BF16 = mybir.dt.bfloat16
I32 = mybir.dt.int32
I16 = mybir.dt.int16
AF = mybir.ActivationFunctionType
OP = mybir.AluOpType

# Problem constants
L0, L1, L2 = 32, 16, 8
NS = 128
NB = 64
CUTOFF = 7.0
N_NODES = 50000
N_EDGES = 400000
NODE_DIM = 120
NCORES = 8

BLK = 512             # edges per block
SUB = 4               # 128-edge sub-tiles per block
P = 128
KTP = 1344            # 1024 + 256 + 64 contraction size
KPAD = 1408           # padded to 11 chunks of 128
NCHUNK = 11
RSQRT_MAGIC = 0x5F3759DF
NPAIR = N_NODES // 2  # 25000
XR = 128              # bf16 units per node row (120 nodes + 6 pos-halves + 2 pad)

E_CORE = N_EDGES // NCORES                      # 50000
ECLS = 13312                                    # padded edges per parity class
NBLK_CLS = ECLS // BLK                          # 26
NBLK = 4 * NBLK_CLS                             # 104
E_PAD = NBLK * BLK                              # 53248
GROUP = 8                                       # blocks per act-table phase group

# cos(pi/2 * sqrt(t)) Taylor coefficients, t = min(d^2/49, 1)
ENV_A = (
    1.0,
    -1.2337005500358182,
    0.25366950654487275,
    -0.020863473217859734,
    0.0009192394784838294,
    -2.5171984603292395e-05,
    4.492184960014096e-07,
)

_compiled = None


def _patch_walrus_dge_levels():
    """This walrus build compiles with DynamicDMA disabled by default, which
    makes dynamic-offset DMAs crash the exec unit. Append the full
    --dge-levels set to every walrus invocation."""
    import concourse.bass_utils as _bu

    if getattr(_bu, "_dge_patched", False):
        return
    orig = _bu.run_command

    def patched(argv, **kw):
        if argv and "walrus_driver" in str(argv[0]) and not any(
            "dge-levels" in str(a) for a in argv
        ):
            argv = list(argv) + [
                "--dge-levels=io,spill_reload,scalar_dynamic_offset,"
                "vector_dynamic_offsets,dynamic_size,dst_reduce,transpose"
            ]
        return orig(argv, **kw)

    _bu.run_command = patched
    _bu._dge_patched = True


_patch_walrus_dge_levels()


def _patch_drain_and_barrier():
    """The final Tile drain runs on the SP engine, whose Drain lowering in this
    walrus build has no free sync-wait slots (its HWDGE queue waits fill them).
    Hoist the tile-clock waits onto dedicated nop instructions emitted just
    before the drain, one wait per nop."""
    if getattr(tile.TileContext, "_dab_patched", False):
        return

    def patched(self, tick_clock, wait_clock):
        nc = self.nc
        nops = [nc.sync.nop() for _ in range(32)]
        drain_inst = nc.sync.drain()
        from concourse.tile import ScopedClock

        wait_clock.add_sem_waits(
            drain_inst.ins, ScopedClock({None: tick_clock.global_clock})
        )
        si = drain_inst.ins.sync_info
        waits = list(si.on_wait) if si and si.on_wait else []
        if waits:
            assert len(waits) <= len(nops), f"{len(waits)} waits > nop slots"
            si.on_wait = []
            for w, n in zip(waits, nops):
                n.ins.sync_info = mybir.SyncInfo(on_wait=[w], on_update=[])

        nc.all_engine_barrier()
        assert self.sems is not None
        popped = nc._tile_sem_poison_stack.pop()
        assert popped is self._sem_poison
        nc.clear_and_free_semaphores(list(self.sems.allocated().values()))
        nc.all_engine_barrier()

    tile.TileContext._drain_and_barrier = patched
    tile.TileContext._dab_patched = True


_patch_drain_and_barrier()


def _newton_rsqrt(nc, pool, u, n, magic_t, tag):
    """rsqrt(u) for u[:, :n] > 0 on the VectorEngine (no ScalarE table)."""
    bits = pool.tile([P, n], I32, tag=f"{tag}_b")
    nc.vector.tensor_copy(out=bits[:].bitcast(F32), in_=u)  # raw bit copy
    nc.vector.tensor_scalar(
        out=bits[:], in0=bits[:], scalar1=1, scalar2=None,
        op0=OP.arith_shift_right,
    )
    yb = pool.tile([P, n], I32, tag=f"{tag}_y")
    nc.vector.tensor_tensor(
        out=yb[:], in0=magic_t[:, 0:1].to_broadcast([P, n]), in1=bits[:],
        op=OP.subtract,
    )
    y = yb[:].bitcast(F32)
    t1 = pool.tile([P, n], F32, tag=f"{tag}_t1")
    for _ in range(3):
        nc.vector.tensor_mul(t1[:], y, y)
        nc.vector.tensor_mul(t1[:], t1[:], u)
        nc.vector.tensor_scalar(
            out=t1[:], in0=t1[:], scalar1=-0.5, scalar2=1.5, op0=OP.mult, op1=OP.add,
        )
        nc.vector.tensor_mul(y, y, t1[:])
    return yb


def _build(nblocks: int):
    nc = bacc.Bacc("TRN2", target_bir_lowering=False, debug=False)

    nodes_pair = nc.dram_tensor("nodes_pair", (NPAIR, 2 * XR), BF16, kind="ExternalInput").ap()
    xw16 = nc.dram_tensor("xw16", (nblocks, P, 64), I16, kind="ExternalInput").ap()
    geo12 = nc.dram_tensor("geo12", (nblocks * BLK, 12), F32, kind="ExternalInput").ap()
    wflat = nc.dram_tensor("wflat", (KPAD, NS), BF16, kind="ExternalInput").ap()
    dfw1 = nc.dram_tensor("dfw1", (NB, 128), BF16, kind="ExternalInput").ap()
    dfb1 = nc.dram_tensor("dfb1", (1, 128), BF16, kind="ExternalInput").ap()
    dfw2gb = nc.dram_tensor("dfw2gb", (128, 256), BF16, kind="ExternalInput").ap()
    dfb2gb = nc.dram_tensor("dfb2gb", (1, 256), BF16, kind="ExternalInput").ap()
    mlpw1 = nc.dram_tensor("mlpw1", (128, 512), BF16, kind="ExternalInput").ap()
    mlpb1 = nc.dram_tensor("mlpb1", (1, 512), BF16, kind="ExternalInput").ap()
    w2row = nc.dram_tensor("w2row", (1, 512), BF16, kind="ExternalInput").ap()
    b2sc = nc.dram_tensor("b2sc", (1, 1), F32, kind="ExternalInput").ap()
    offs = nc.dram_tensor("offs", (1, NB), F32, kind="ExternalInput").ap()
    out = nc.dram_tensor("out", (nblocks * BLK,), F32, kind="ExternalOutput").ap()

    width = CUTOFF / (NB - 1)
    coeff = 0.5 / (width * width)
    sqc = float(np.sqrt(coeff))

    XGBUFS = GROUP + 3

    with tile.TileContext(nc) as tc:
        with (
            tc.tile_pool(name="const", bufs=1) as constp,
            tc.tile_pool(name="xgp", bufs=XGBUFS) as xgp,
            tc.tile_pool(name="dembp", bufs=XGBUFS) as dembp,
            tc.tile_pool(name="io", bufs=3) as iop,
            tc.tile_pool(name="geo", bufs=3) as geop,
            tc.tile_pool(name="pfeat", bufs=2) as pfp,
            tc.tile_pool(name="trsb", bufs=3) as trsbp,
            tc.tile_pool(name="work", bufs=3) as workp,
            tc.tile_pool(name="acc", bufs=2) as accp,
            tc.tile_pool(name="ps_tr", bufs=2, space="PSUM") as ps_tr,
            tc.tile_pool(name="ps_mm", bufs=2, space="PSUM") as ps_mm,
            tc.tile_pool(name="ps_h", bufs=1, space="PSUM") as ps_h,
            tc.tile_pool(name="ps_df", bufs=1, space="PSUM") as ps_df,
            tc.tile_pool(name="ps_g", bufs=2, space="PSUM") as ps_g,
        ):
            # ---- resident constants ----
            identb = constp.tile([P, P], BF16)
            make_identity(nc, identb[:])
            magic_t = constp.tile([P, 1], I32)
            nc.vector.memset(magic_t[:], RSQRT_MAGIC)
            ones_row = constp.tile([1, P], BF16)
            nc.vector.memset(ones_row[:], 1.0)

            w_sb = constp.tile([P, NCHUNK, P], BF16)
            nc.sync.dma_start(out=w_sb[:], in_=wflat.rearrange("(c p) w -> p c w", p=P))
            dfw1_sb = constp.tile([NB, 128], BF16)
            nc.sync.dma_start(out=dfw1_sb[:], in_=dfw1)
            dfw2gb_sb = constp.tile([128, 256], BF16)
            nc.sync.dma_start(out=dfw2gb_sb[:], in_=dfw2gb)
            mlpw1_sb = constp.tile([128, 512], BF16)
            nc.sync.dma_start(out=mlpw1_sb[:], in_=mlpw1)
            dfb1_sb = constp.tile([1, 128], BF16)
            nc.sync.dma_start(out=dfb1_sb[:], in_=dfb1)
            dfb2gb_sb = constp.tile([1, 256], BF16)
            nc.sync.dma_start(out=dfb2gb_sb[:], in_=dfb2gb)
            mlpb1_sb = constp.tile([1, 512], BF16)
            nc.sync.dma_start(out=mlpb1_sb[:], in_=mlpb1)
            w2rep_sb = constp.tile([P, 512], BF16)
            nc.sync.dma_start(out=w2rep_sb[:], in_=w2row.to_broadcast([P, 512]))
            b2_sb = constp.tile([P, 1], F32)
            nc.sync.dma_start(out=b2_sb[:], in_=b2sc.to_broadcast([P, 1]))
            offs_sb = constp.tile([P, NB], F32)
            nc.sync.dma_start(out=offs_sb[:], in_=offs.to_broadcast([P, NB]))

            xg_tiles = {}
            demb_tiles = {}
            n_ptb = 0

            groups = [range(g, min(g + GROUP, nblocks)) for g in range(0, nblocks, GROUP)]
            for blocks in groups:
                # ======== Phase A: gather + geometry + RBF (exp table) ========
                for b in blocks:
                    cls = b // NBLK_CLS
                    ps, pd = (cls >> 1) & 1, cls & 1
                    e0 = b * BLK
                    sl = slice(e0, e0 + BLK)

                    xw = iop.tile([P, 64], I16, tag="xw")
                    nc.sync.dma_start(out=xw[:], in_=xw16[b])
                    geo = iop.tile([P, SUB, 12], F32, tag="geo")
                    nc.sync.dma_start(out=geo[:], in_=geo12[sl, :].rearrange("(s p) j -> p s j", p=P))

                    xg = xgp.tile([P, 2 * SUB, 2 * XR], BF16, tag="xg")
                    nc.gpsimd.dma_gather(
                        out_ap=xg[:], in_ap=nodes_pair[:, :], idxs_ap=xw[:],
                        num_idxs=2 * BLK, num_idxs_reg=2 * BLK, elem_size=2 * XR,
                    )
                    xg_tiles[b] = xg

                    # fp32 positions bit-packed into the bf16 rows
                    p1 = xg[:, 0:SUB, ps * XR + 120 : ps * XR + 126].bitcast(F32)
                    p2 = xg[:, SUB : 2 * SUB, pd * XR + 120 : pd * XR + 126].bitcast(F32)

                    # tvec[p,s,j] = sum_i esh[p,s,i] * bcell[p,s,3i+j]
                    tvp = geop.tile([P, SUB, 3, 3], F32, tag="tvp")
                    nc.vector.tensor_tensor(
                        out=tvp[:],
                        in0=geo[:, :, 0:3].unsqueeze(3).to_broadcast([P, SUB, 3, 3]),
                        in1=geo[:, :, 3:12].rearrange("p s (i j) -> p s i j", j=3),
                        op=OP.mult,
                    )
                    tv = geop.tile([P, SUB, 3], F32, tag="tv")
                    nc.vector.reduce_sum(
                        out=tv[:], in_=tvp[:].transpose([0, 1, 3, 2]), axis=mybir.AxisListType.X,
                    )
                    rv = geop.tile([P, SUB, 3], F32, tag="rv")
                    nc.vector.tensor_sub(rv[:], p2, p1)
                    nc.vector.tensor_add(rv[:], rv[:], tv[:])
                    rv2 = geop.tile([P, SUB, 3], F32, tag="rv2")
                    nc.vector.tensor_mul(rv2[:], rv[:], rv[:])
                    d2 = geop.tile([P, SUB], F32, tag="d2")
                    nc.vector.reduce_sum(out=d2[:], in_=rv2[:], axis=mybir.AxisListType.X)
                    nc.vector.tensor_scalar(
                        out=d2[:], in0=d2[:], scalar1=1e-12, scalar2=None, op0=OP.max,
                    )
                    ry = _newton_rsqrt(nc, geop, d2[:], SUB, magic_t, "rsq")
                    dist = geop.tile([P, SUB], F32, tag="dist")
                    nc.vector.tensor_mul(dist[:], d2[:], ry[:].bitcast(F32))

                    # envelope: env = p(t)^2, t = min(d2/49, 1)
                    tgeo = geop.tile([P, SUB], F32, tag="tgeo")
                    nc.vector.tensor_scalar(
                        out=tgeo[:], in0=d2[:], scalar1=1.0 / 49.0, scalar2=1.0,
                        op0=OP.mult, op1=OP.min,
                    )
                    envr = geop.tile([P, SUB], F32, tag="envr")
                    nc.vector.tensor_scalar(
                        out=envr[:], in0=tgeo[:], scalar1=ENV_A[6], scalar2=None, op0=OP.mult,
                    )
                    for k in range(5, 0, -1):
                        nc.vector.scalar_tensor_tensor(
                            out=envr[:], in0=envr[:], scalar=ENV_A[k], in1=tgeo[:],
                            op0=OP.add, op1=OP.mult,
                        )
                    nc.vector.tensor_scalar(
                        out=envr[:], in0=envr[:], scalar1=ENV_A[0], scalar2=None, op0=OP.add,
                    )
                    env = geop.tile([P, SUB], F32, tag="env")
                    nc.vector.tensor_mul(env[:], envr[:], envr[:])

                    # rbf then demb = rbf * env
                    rb = geop.tile([P, SUB, NB], F32, tag="rb")
                    nc.vector.tensor_tensor(
                        out=rb[:],
                        in0=offs_sb[:].unsqueeze(1).to_broadcast([P, SUB, NB]),
                        in1=dist[:].unsqueeze(2).to_broadcast([P, SUB, NB]),
                        op=OP.subtract,
                    )
                    nc.scalar.activation(rb[:], rb[:], AF.Square, scale=sqc)
                    nc.scalar.activation(rb[:], rb[:], AF.Exp, scale=-1.0)
                    demb = dembp.tile([P, SUB, NB], BF16, tag="demb")
                    nc.vector.tensor_tensor(
                        out=demb[:], in0=rb[:],
                        in1=env[:].unsqueeze(2).to_broadcast([P, SUB, NB]),
                        op=OP.mult,
                    )
                    demb_tiles[b] = demb

                # ======== Phase B: TP + LN + dfilter + MLP (silu table) ========
                for b in blocks:
                    cls = b // NBLK_CLS
                    ps, pd = (cls >> 1) & 1, cls & 1
                    e0 = b * BLK
                    sl = slice(e0, e0 + BLK)
                    xg = xg_tiles.pop(b)
                    demb = demb_tiles.pop(b)

                    x1 = xg[:, 0:SUB, ps * XR : ps * XR + 120]
                    x2 = xg[:, SUB : 2 * SUB, pd * XR : pd * XR + 120]

                    psmix = ps_mm.tile([P, SUB, NS], F32, tag="psmix")
                    muv = geop.tile([P, SUB], F32, tag="muv")
                    varv = geop.tile([P, SUB], F32, tag="varv")

                    # ---- pass 1: tensor product per sub-tile ----
                    for s in range(SUB):
                        ptb = pfp.tile([P, KPAD], BF16, tag="ptb")
                        if n_ptb < 2:
                            nc.vector.memset(ptb[:, KTP:KPAD], 0.0)
                            n_ptb += 1
                        a1 = x1[:, s, 0:L0]
                        a2 = x2[:, s, 0:L0]
                        nc.vector.tensor_tensor(
                            out=ptb[:, 0:1024].rearrange("p (u v) -> p u v", v=L0),
                            in0=a1.unsqueeze(2).to_broadcast([P, L0, L0]),
                            in1=a2.unsqueeze(1).to_broadcast([P, L0, L0]),
                            op=OP.mult,
                        )
                        b1 = x1[:, s, 32:80].rearrange("p (u m) -> p u m", m=3)
                        b2 = x2[:, s, 32:80].rearrange("p (u m) -> p u m", m=3)
                        pb = workp.tile([P, L1, L1, 3], BF16, tag="pb")
                        nc.vector.tensor_tensor(
                            out=pb[:],
                            in0=b1.unsqueeze(2).to_broadcast([P, L1, L1, 3]),
                            in1=b2.unsqueeze(1).to_broadcast([P, L1, L1, 3]),
                            op=OP.mult,
                        )
                        with nc.allow_low_precision(reason="3-term bf16 reduce"):
                            nc.vector.reduce_sum(
                                out=ptb[:, 1024:1280].rearrange("p (u v) -> p u v", v=L1),
                                in_=pb[:], axis=mybir.AxisListType.X,
                            )
                        c1 = x1[:, s, 80:120].rearrange("p (u m) -> p u m", m=5)
                        c2 = x2[:, s, 80:120].rearrange("p (u m) -> p u m", m=5)
                        pc = workp.tile([P, L2, L2, 5], BF16, tag="pc")
                        nc.vector.tensor_tensor(
                            out=pc[:],
                            in0=c1.unsqueeze(2).to_broadcast([P, L2, L2, 5]),
                            in1=c2.unsqueeze(1).to_broadcast([P, L2, L2, 5]),
                            op=OP.mult,
                        )
                        with nc.allow_low_precision(reason="5-term bf16 reduce"):
                            nc.vector.reduce_sum(
                                out=ptb[:, 1280:1344].rearrange("p (u v) -> p u v", v=L2),
                                in_=pc[:], axis=mybir.AxisListType.X,
                            )

                        # transposes in groups of <=4 chunks -> one PSUM bank,
                        # one batched PSUM->SBUF copy per group
                        for g, chunks in enumerate(((0, 1, 2, 3), (4, 5, 6, 7), (8, 9, 10))):
                            ptp = ps_tr.tile([P, 4, P], BF16, tag="ptp")
                            for j, c in enumerate(chunks):
                                nc.tensor.transpose(
                                    ptp[:, j, :], ptb[:, c * P : (c + 1) * P], identb[:]
                                )
                            pts = trsbp.tile([P, 4, P], BF16, tag="pts")
                            ncopy = len(chunks)
                            if g == 1:
                                nc.scalar.copy(pts[:, 0:ncopy, :], ptp[:, 0:ncopy, :])
                            else:
                                nc.vector.tensor_copy(pts[:, 0:ncopy, :], ptp[:, 0:ncopy, :])
                            for j, c in enumerate(chunks):
                                nc.tensor.matmul(
                                    psmix[:, s, :], lhsT=pts[:, j, :], rhs=w_sb[:, c, :],
                                    start=(c == 0), stop=(c == NCHUNK - 1),
                                )

                        stats = geop.tile([P, 6], F32, tag="stats")
                        nc.vector.bn_stats(out=stats[:], in_=psmix[:, s, :])
                        mv = geop.tile([P, 2], F32, tag="mv")
                        nc.vector.bn_aggr(out=mv[:], in_=stats[:])
                        nc.vector.tensor_copy(out=muv[:, s : s + 1], in_=mv[:, 0:1])
                        nc.vector.tensor_copy(out=varv[:, s : s + 1], in_=mv[:, 1:2])

                    # ---- block-level LN rstd ----
                    nc.vector.tensor_scalar(
                        out=varv[:], in0=varv[:], scalar1=1e-5, scalar2=None, op0=OP.add,
                    )
                    ryl = _newton_rsqrt(nc, geop, varv[:], SUB, magic_t, "lnr")
                    rstd = ryl[:].bitcast(F32)
                    tb = geop.tile([P, SUB], F32, tag="tb")
                    nc.vector.tensor_mul(tb[:], muv[:], rstd)
                    nc.vector.tensor_scalar(
                        out=tb[:], in0=tb[:], scalar1=-1.0, scalar2=None, op0=OP.mult,
                    )

                    acc = accp.tile([P, SUB], F32, tag="acc")

                    # ---- pass 2: LN apply + dfilter + final MLP ----
                    for s in range(SUB):
                        ynorm = workp.tile([P, NS], BF16, tag="ynorm")
                        nc.scalar.activation(
                            ynorm[:], psmix[:, s, :], AF.Identity,
                            bias=tb[:, s : s + 1], scale=rstd[:, s : s + 1],
                        )

                        dT_ps = ps_tr.tile([P, 4, P], BF16, tag="ptp")
                        nc.tensor.transpose(dT_ps[0:NB, 0, :], demb[:, s, :], identb[:])
                        dT = trsbp.tile([NB, P], BF16, tag="dT")
                        nc.scalar.copy(dT[:], dT_ps[0:NB, 0, :])
                        ph = ps_h.tile([P, 128], F32, tag="ph")
                        nc.tensor.matmul(ph[:], lhsT=dT[:], rhs=dfw1_sb[:], start=True, stop=False)
                        nc.tensor.matmul(ph[:], lhsT=ones_row[:], rhs=dfb1_sb[:], start=False, stop=True)
                        sact = workp.tile([P, 128], BF16, tag="sact")
                        nc.scalar.activation(sact[:], ph[:], AF.Silu)
                        sT_ps = ps_tr.tile([P, 4, P], BF16, tag="ptp")
                        nc.tensor.transpose(sT_ps[:, 0, :], sact[:], identb[:])
                        sT = trsbp.tile([P, P], BF16, tag="sT")
                        nc.vector.tensor_copy(sT[:], sT_ps[:, 0, :])
                        pdf = ps_df.tile([P, 256], F32, tag="pdf")
                        nc.tensor.matmul(pdf[:], lhsT=sT[:], rhs=dfw2gb_sb[:], start=True, stop=False)
                        nc.tensor.matmul(pdf[:], lhsT=ones_row[:], rhs=dfb2gb_sb[:], start=False, stop=True)

                        rg = workp.tile([P, 128], BF16, tag="rg")
                        nc.vector.tensor_mul(rg[:], ynorm[:], pdf[:, 0:128])
                        nc.vector.tensor_add(rg[:], rg[:], pdf[:, 128:256])

                        rT_ps = ps_tr.tile([P, 4, P], BF16, tag="ptp")
                        nc.tensor.transpose(rT_ps[:, 0, :], rg[:], identb[:])
                        rT = trsbp.tile([P, P], BF16, tag="rT")
                        nc.scalar.copy(rT[:], rT_ps[:, 0, :])
                        pg2 = ps_g.tile([P, 512], F32, tag="pg")
                        nc.tensor.matmul(pg2[:], lhsT=rT[:], rhs=mlpw1_sb[:], start=True, stop=False)
                        nc.tensor.matmul(pg2[:], lhsT=ones_row[:], rhs=mlpb1_sb[:], start=False, stop=True)
                        gact = workp.tile([P, 512], BF16, tag="gact")
                        nc.scalar.activation(gact[:], pg2[:], AF.Silu)
                        scr = workp.tile([P, 512], BF16, tag="scr")
                        nc.vector.tensor_mul(scr[:], gact[:], w2rep_sb[:])
                        nc.vector.reduce_sum(
                            out=acc[:, s : s + 1], in_=scr[:], axis=mybir.AxisListType.X,
                        )

                    nc.vector.tensor_scalar(
                        out=acc[:], in0=acc[:], scalar1=b2_sb[:, 0:1], scalar2=None,
                        op0=OP.add,
                    )
                    nc.sync.dma_start(out=out[sl].rearrange("(s p) -> p s", p=P), in_=acc[:])

    nc.compile()
    return nc


def _get_compiled():
    global _compiled
    if _compiled is None:
        _compiled = _build(NBLK)
    return _compiled


def _wrap16(idx_block):
    """int array [512] -> dma_gather wrapped int16 layout [128, 32]
    (index j at [j%16, j//16], replicated across the 8 gpsimd cores)."""
    w = idx_block.astype(np.int16).reshape(-1, 16).T  # [16, n/16]
    return np.tile(w, (8, 1))


def _prep(inputs):
    nodes = np.asarray(inputs["nodes"], np.float32)
    edge_index = np.asarray(inputs["edge_index"]).astype(np.int64)
    graph_batch = np.asarray(inputs["graph_batch"]).astype(np.int64)
    cell = np.asarray(inputs["cell"], np.float32).reshape(32, 9)
    edge_shift = np.asarray(inputs["edge_shift"], np.float32)
    pos = np.asarray(inputs["pos"], np.float32)

    # bf16 pair-row node table with fp32 pos bit-packed at units 120:126
    row_u16 = np.zeros((N_NODES, XR), np.uint16)
    row_u16[:, 0:NODE_DIM] = nodes.astype(ml_dtypes.bfloat16).view(np.uint16)
    row_u16[:, 120:126] = pos.view(np.uint16).reshape(N_NODES, 6)
    nodes_pair = row_u16.reshape(NPAIR, 2 * XR).view(ml_dtypes.bfloat16)

    alpha = 1.0 / np.sqrt(float(L0 * L0 + L1 * L1 + L2 * L2))
    w0 = np.asarray(inputs["W0"], np.float32).reshape(L0 * L0, NS) * alpha
    w1 = np.asarray(inputs["W1"], np.float32).reshape(L1 * L1, NS) * (alpha / np.sqrt(3.0))
    w2 = np.asarray(inputs["W2"], np.float32).reshape(L2 * L2, NS) * (alpha / np.sqrt(5.0))
    wflat = np.zeros((KPAD, NS), np.float32)
    wflat[0:1024] = w0
    wflat[1024:1280] = w1
    wflat[1280:1344] = w2

    ln_g = np.asarray(inputs["ln_g"], np.float32)
    ln_b = np.asarray(inputs["ln_b"], np.float32)
    df_w2 = np.asarray(inputs["df_w2"], np.float32)
    df_b2 = np.asarray(inputs["df_b2"], np.float32)
    dfw2gb = np.concatenate([df_w2 * ln_g[None, :], df_w2 * ln_b[None, :]], axis=1)
    dfb2gb = np.concatenate([df_b2 * ln_g, df_b2 * ln_b])[None, :]

    bf = lambda a: np.ascontiguousarray(a).astype(ml_dtypes.bfloat16)

    common = {
        "nodes_pair": nodes_pair,
        "wflat": bf(wflat),
        "dfw1": bf(np.asarray(inputs["df_w1"], np.float32)),
        "dfb1": bf(np.asarray(inputs["df_b1"], np.float32)[None, :]),
        "dfw2gb": bf(dfw2gb),
        "dfb2gb": bf(dfb2gb),
        "mlpw1": bf(np.asarray(inputs["mlp_w1"], np.float32)),
        "mlpb1": bf(np.asarray(inputs["mlp_b1"], np.float32)[None, :]),
        "w2row": bf(np.asarray(inputs["mlp_w2"], np.float32).T),
        "b2sc": np.asarray(inputs["mlp_b2"], np.float32).reshape(1, 1),
        "offs": np.linspace(0.0, CUTOFF, NB, dtype=np.float32)[None, :],
    }

    in_maps = []
    outmaps = []
    for c in range(NCORES):
        lo, hi = c * E_CORE, (c + 1) * E_CORE
        src = edge_index[0, lo:hi]
        dst = edge_index[1, lo:hi]
        esh = edge_shift[lo:hi]
        key = ((src & 1) << 1) | (dst & 1)

        srcp = np.zeros(E_PAD, np.int64)
        dstp = np.zeros(E_PAD, np.int64)
        geo = np.zeros((E_PAD, 12), np.float32)
        outmap = np.full(E_PAD, -1, np.int64)
        for cls in range(4):
            idxs = np.nonzero(key == cls)[0]
            n = len(idxs)
            assert n <= ECLS, f"class {cls} overflow: {n} > {ECLS}"
            base = cls * ECLS
            srcp[base : base + n] = src[idxs]
            dstp[base : base + n] = dst[idxs]
            geo[base : base + n, 0:3] = esh[idxs]
            geo[base : base + n, 3:12] = cell[graph_batch[src[idxs]]]
            outmap[base : base + n] = idxs

        xw = np.zeros((NBLK, P, 64), np.int16)
        for b in range(NBLK):
            sb = srcp[b * BLK : (b + 1) * BLK]
            db = dstp[b * BLK : (b + 1) * BLK]
            xw[b, :, 0:32] = _wrap16(sb >> 1)
            xw[b, :, 32:64] = _wrap16(db >> 1)

        m = dict(common)
        m["xw16"] = xw
        m["geo12"] = geo
        in_maps.append(m)
        outmaps.append(outmap)
    return in_maps, outmaps


def _gather_out(res, outmaps):
    full = np.empty((N_EDGES,), np.float32)
    for c in range(NCORES):
        dev = np.asarray(res.results[c]["out"])
        outmap = outmaps[c]
        valid = outmap >= 0
        full[c * E_CORE + outmap[valid]] = dev[valid]
    return full.reshape(N_EDGES, 1)


def kernel(**inputs) -> np.ndarray:
    nc = _get_compiled()
    in_maps, outmaps = _prep(inputs)
    res = run_bass_kernel_spmd(nc, in_maps, core_ids=list(range(NCORES)))
    return _gather_out(res, outmaps)


# revision 9
# speedup vs baseline: 2.5367x; 1.6528x over previous
"""Trainium2 Bass kernel for nn_ExchangeBlock (GNN message passing / e3nn-style
tensor-product edge block), SPMD across 8 NeuronCores.

Sharding: edges across the 8 cores; node features and params replicated.

v4 design notes:
- Edges are host-sorted into 4 parity classes (src&1, dst&1) so the pair-row
  parity select becomes a compile-time AP slice: no masks, no predicated
  copies, no gpsimd copy traffic at all.
- ONE dma_gather per block: the node table is bf16 pair rows (512B) with the
  fp32 position bit-packed into units 120:126 of each row, so geometry and
  the tensor product share the same gathered tile.  cell[graph_batch[src]]
  is a host-side index prep (like the baseline's graph_batch[src]) and is
  streamed per edge together with edge_shift.
- Blocks run in groups of 8 with two ScalarE activation-table phases per
  group (exp set: RBF; silu set: MLPs).  The per-edge scalar chain (Newton
  rsqrt, cutoff envelope as an exact degree-6 polynomial in d^2, RBF, demb)
  is batched once per group, so phase A is a handful of wide ops and the
  Square/Exp are single instructions the OoO scheduler cannot shred.
- The TP runs as outer-product features P[e,1344] built on DVE (bf16), PE
  transposes of P chunks (one full PSUM bank per 8 chunks), and accumulated
  128x128 matmuls against pre-scaled flattened weights.  The 1o/2e paths
  build m-major product tiles and reduce with 2 contiguous bf16 adds
  instead of a slow innermost-3 reduce.
- LN stats via one PSUM reduce + a batched ScalarE Square + one bf16
  reduce; the final w2 contraction fuses multiply+reduce into one
  scalar_tensor_tensor with accum_out.
- All MLP biases in this problem are exactly zero; _prep detects that and
  compiles the bias-free variant (rank-1 PE bias matmuls otherwise).
"""

import sys

sys.path.insert(0, "/opt/trn_rl_repo")

import numpy as np
import ml_dtypes

import concourse.bass as bass
import concourse.mybir as mybir
import concourse.tile as tile
from concourse import bacc
from concourse.bass_utils import run_bass_kernel_spmd
from concourse.masks import make_identity

F32 = mybir.dt.float32
BF16 = mybir.dt.bfloat16
I32 = mybir.dt.int32
I16 = mybir.dt.int16
AF = mybir.ActivationFunctionType
OP = mybir.AluOpType

# Problem constants
L0, L1, L2 = 32, 16, 8
NS = 128
NB = 64
CUTOFF = 7.0
N_NODES = 50000
N_EDGES = 400000
NODE_DIM = 120
NCORES = 8

BLK = 512             # edges per block
SUB = 4               # 128-edge sub-tiles per block
P = 128
KTP = 1344            # 1024 + 256 + 64 contraction size
KPAD = 1408           # padded to 11 chunks of 128
NCHUNK = 11
RSQRT_MAGIC = 0x5F3759DF
NPAIR = N_NODES // 2  # 25000
XR = 128              # bf16 units per node row (120 nodes + 6 pos-halves + 2 pad)

E_CORE = N_EDGES // NCORES                      # 50000
ECLS = 13312                                    # padded edges per parity class
NBLK_CLS = ECLS // BLK                          # 26
NBLK = 4 * NBLK_CLS                             # 104
E_PAD = NBLK * BLK                              # 53248
GROUP = 8                                       # blocks per act-table phase group
GS = GROUP * SUB                                # 32 sub-tiles per group

# cos(pi/2 * sqrt(t)) Taylor coefficients, t = min(d^2/49, 1)
ENV_A = (
    1.0,
    -1.2337005500358182,
    0.25366950654487275,
    -0.020863473217859734,
    0.0009192394784838294,
    -2.5171984603292395e-05,
    4.492184960014096e-07,
)

_compiled = {}


def _patch_walrus_dge_levels():
    """This walrus build compiles with DynamicDMA disabled by default, which
    makes dynamic-offset DMAs crash the exec unit. Append the full
    --dge-levels set to every walrus invocation."""
    import concourse.bass_utils as _bu

    if getattr(_bu, "_dge_patched", False):
        return
    orig = _bu.run_command

    def patched(argv, **kw):
        if argv and "walrus_driver" in str(argv[0]) and not any(
            "dge-levels" in str(a) for a in argv
        ):
            argv = list(argv) + [
                "--dge-levels=io,spill_reload,scalar_dynamic_offset,"
                "vector_dynamic_offsets,dynamic_size,dst_reduce,transpose"
            ]
        return orig(argv, **kw)

    _bu.run_command = patched
    _bu._dge_patched = True


_patch_walrus_dge_levels()


def _patch_drain_and_barrier():
    """The final Tile drain runs on the SP engine, whose Drain lowering in this
    walrus build has no free sync-wait slots (its HWDGE queue waits fill them).
    Hoist the tile-clock waits onto dedicated nop instructions emitted just
    before the drain, one wait per nop."""
    if getattr(tile.TileContext, "_dab_patched", False):
        return

    def patched(self, tick_clock, wait_clock):
        nc = self.nc
        nops = [nc.sync.nop() for _ in range(32)]
        drain_inst = nc.sync.drain()
        from concourse.tile import ScopedClock

        wait_clock.add_sem_waits(
            drain_inst.ins, ScopedClock({None: tick_clock.global_clock})
        )
        si = drain_inst.ins.sync_info
        waits = list(si.on_wait) if si and si.on_wait else []
        if waits:
            assert len(waits) <= len(nops), f"{len(waits)} waits > nop slots"
            si.on_wait = []
            for w, n in zip(waits, nops):
                n.ins.sync_info = mybir.SyncInfo(on_wait=[w], on_update=[])

        nc.all_engine_barrier()
        assert self.sems is not None
        popped = nc._tile_sem_poison_stack.pop()
        assert popped is self._sem_poison
        nc.clear_and_free_semaphores(list(self.sems.allocated().values()))
        nc.all_engine_barrier()

    tile.TileContext._drain_and_barrier = patched
    tile.TileContext._dab_patched = True


_patch_drain_and_barrier()


def _newton_rsqrt(nc, pool, u, n, magic_t, tag):
    """rsqrt(u) for u[:, :n] > 0 on the VectorEngine (no ScalarE table)."""
    bits = pool.tile([P, n], I32, tag=f"{tag}_b")
    nc.vector.tensor_copy(out=bits[:].bitcast(F32), in_=u)  # raw bit copy
    nc.vector.tensor_scalar(
        out=bits[:], in0=bits[:], scalar1=1, scalar2=None,
        op0=OP.arith_shift_right,
    )
    yb = pool.tile([P, n], I32, tag=f"{tag}_y")
    nc.vector.tensor_tensor(
        out=yb[:], in0=magic_t[:, 0:1].to_broadcast([P, n]), in1=bits[:],
        op=OP.subtract,
    )
    y = yb[:].bitcast(F32)
    t1 = pool.tile([P, n], F32, tag=f"{tag}_t1")
    for _ in range(3):
        nc.vector.tensor_mul(t1[:], y, y)
        nc.vector.tensor_mul(t1[:], t1[:], u)
        nc.vector.tensor_scalar(
            out=t1[:], in0=t1[:], scalar1=-0.5, scalar2=1.5, op0=OP.mult, op1=OP.add,
        )
        nc.vector.tensor_mul(y, y, t1[:])
    return yb


def _build(nblocks: int, zero_bias: bool):
    nc = bacc.Bacc("TRN2", target_bir_lowering=False, debug=False)

    nodes_pair = nc.dram_tensor("nodes_pair", (NPAIR, 2 * XR), BF16, kind="ExternalInput").ap()
    xw16 = nc.dram_tensor("xw16", (nblocks, P, 64), I16, kind="ExternalInput").ap()
    geo12 = nc.dram_tensor("geo12", (nblocks * BLK, 12), F32, kind="ExternalInput").ap()
    wflat = nc.dram_tensor("wflat", (KPAD, NS), BF16, kind="ExternalInput").ap()
    dfw1 = nc.dram_tensor("dfw1", (NB, 128), BF16, kind="ExternalInput").ap()
    dfb1 = nc.dram_tensor("dfb1", (1, 128), BF16, kind="ExternalInput").ap()
    dfw2gb = nc.dram_tensor("dfw2gb", (128, 256), BF16, kind="ExternalInput").ap()
    dfb2gb = nc.dram_tensor("dfb2gb", (1, 256), BF16, kind="ExternalInput").ap()
    mlpw1 = nc.dram_tensor("mlpw1", (128, 512), BF16, kind="ExternalInput").ap()
    mlpb1 = nc.dram_tensor("mlpb1", (1, 512), BF16, kind="ExternalInput").ap()
    w2row = nc.dram_tensor("w2row", (1, 512), BF16, kind="ExternalInput").ap()
    b2sc = nc.dram_tensor("b2sc", (1, 1), F32, kind="ExternalInput").ap()
    offs = nc.dram_tensor("offs", (1, NB), F32, kind="ExternalInput").ap()
    out = nc.dram_tensor("out", (nblocks * BLK,), F32, kind="ExternalOutput").ap()

    width = CUTOFF / (NB - 1)
    coeff = 0.5 / (width * width)
    sqc = float(np.sqrt(coeff))

    XGBUFS = GROUP + 3

    with tile.TileContext(nc) as tc:
        with (
            tc.tile_pool(name="const", bufs=1) as constp,
            tc.tile_pool(name="xgp", bufs=XGBUFS) as xgp,
            tc.tile_pool(name="grp", bufs=2) as grpp,
            tc.tile_pool(name="io", bufs=3) as iop,
            tc.tile_pool(name="geo", bufs=3) as geop,
            tc.tile_pool(name="pfeat", bufs=2) as pfp,
            tc.tile_pool(name="trsb", bufs=3) as trsbp,
            tc.tile_pool(name="work", bufs=3) as workp,
            tc.tile_pool(name="acc", bufs=2) as accp,
            tc.tile_pool(name="ps_t8", bufs=2, space="PSUM") as ps_t8,
            tc.tile_pool(name="ps_mm", bufs=2, space="PSUM") as ps_mm,
            tc.tile_pool(name="ps_hd", bufs=1, space="PSUM") as ps_hd,
            tc.tile_pool(name="ps_g", bufs=2, space="PSUM") as ps_g,
        ):
            # ---- resident constants ----
            identb = constp.tile([P, P], BF16)
            make_identity(nc, identb[:])
            magic_t = constp.tile([P, 1], I32)
            nc.vector.memset(magic_t[:], RSQRT_MAGIC)
            ones_row = constp.tile([1, P], BF16)
            nc.vector.memset(ones_row[:], 1.0)

            w_sb = constp.tile([P, NCHUNK, P], BF16)
            nc.sync.dma_start(out=w_sb[:], in_=wflat.rearrange("(c p) w -> p c w", p=P))
            dfw1_sb = constp.tile([P, 128], BF16)
            nc.sync.dma_start(out=dfw1_sb[0:NB, :], in_=dfw1)
            nc.sync.dma_start(out=dfw1_sb[NB:P, :], in_=dfw1)
            dfw2gb_sb = constp.tile([128, 256], BF16)
            nc.sync.dma_start(out=dfw2gb_sb[:], in_=dfw2gb)
            mlpw1_sb = constp.tile([128, 512], BF16)
            nc.sync.dma_start(out=mlpw1_sb[:], in_=mlpw1)
            w2rep_sb = constp.tile([P, 512], BF16)
            nc.sync.dma_start(out=w2rep_sb[:], in_=w2row.to_broadcast([P, 512]))
            b2_sb = constp.tile([P, 1], F32)
            nc.sync.dma_start(out=b2_sb[:], in_=b2sc.to_broadcast([P, 1]))
            offs_sb = constp.tile([P, NB], F32)
            nc.sync.dma_start(out=offs_sb[:], in_=offs.to_broadcast([P, NB]))
            if not zero_bias:
                dfb1_sb = constp.tile([1, 128], BF16)
                nc.sync.dma_start(out=dfb1_sb[:], in_=dfb1)
                dfb2gb_sb = constp.tile([1, 256], BF16)
                nc.sync.dma_start(out=dfb2gb_sb[:], in_=dfb2gb)
                mlpb1_sb = constp.tile([1, 512], BF16)
                nc.sync.dma_start(out=mlpb1_sb[:], in_=mlpb1)

            xg_tiles = {}
            n_ptb = 0

            groups = [range(g, min(g + GROUP, nblocks)) for g in range(0, nblocks, GROUP)]
            for blocks in groups:
                gn = len(blocks)
                # ======== Phase A: gather + geometry + RBF (exp table) ========
                d2g = grpp.tile([P, GS], F32, tag="d2g")
                for i, b in enumerate(blocks):
                    cls = b // NBLK_CLS
                    ps, pd = (cls >> 1) & 1, cls & 1
                    e0 = b * BLK
                    sl = slice(e0, e0 + BLK)

                    xw = iop.tile([P, 64], I16, tag="xw")
                    nc.sync.dma_start(out=xw[:], in_=xw16[b])
                    geo = iop.tile([P, SUB, 12], F32, tag="geo")
                    nc.sync.dma_start(out=geo[:], in_=geo12[sl, :].rearrange("(s p) j -> p s j", p=P))

                    xg = xgp.tile([P, 2 * SUB, 2 * XR], BF16, tag="xg")
                    nc.gpsimd.dma_gather(
                        out_ap=xg[:], in_ap=nodes_pair[:, :], idxs_ap=xw[:],
                        num_idxs=2 * BLK, num_idxs_reg=2 * BLK, elem_size=2 * XR,
                    )
                    xg_tiles[b] = xg

                    # fp32 positions bit-packed into the bf16 rows
                    p1 = xg[:, 0:SUB, ps * XR + 120 : ps * XR + 126].bitcast(F32)
                    p2 = xg[:, SUB : 2 * SUB, pd * XR + 120 : pd * XR + 126].bitcast(F32)

                    # tvec[p,s,j] = sum_i esh[p,s,i] * bcell[p,s,3i+j]
                    tvp = geop.tile([P, SUB, 3, 3], F32, tag="tvp")
                    nc.vector.tensor_tensor(
                        out=tvp[:],
                        in0=geo[:, :, 0:3].unsqueeze(3).to_broadcast([P, SUB, 3, 3]),
                        in1=geo[:, :, 3:12].rearrange("p s (i j) -> p s i j", j=3),
                        op=OP.mult,
                    )
                    tv = geop.tile([P, SUB, 3], F32, tag="tv")
                    nc.vector.reduce_sum(
                        out=tv[:], in_=tvp[:].transpose([0, 1, 3, 2]), axis=mybir.AxisListType.X,
                    )
                    rv = geop.tile([P, SUB, 3], F32, tag="rv")
                    nc.vector.tensor_sub(rv[:], p2, p1)
                    nc.vector.tensor_add(rv[:], rv[:], tv[:])
                    rv2 = geop.tile([P, SUB, 3], F32, tag="rv2")
                    nc.vector.tensor_mul(rv2[:], rv[:], rv[:])
                    nc.vector.reduce_sum(
                        out=d2g[:, i * SUB : (i + 1) * SUB], in_=rv2[:],
                        axis=mybir.AxisListType.X,
                    )

                ng = gn * SUB
                nc.vector.tensor_scalar(
                    out=d2g[:, 0:ng], in0=d2g[:, 0:ng], scalar1=1e-12, scalar2=None,
                    op0=OP.max,
                )
                ry = _newton_rsqrt(nc, grpp, d2g[:, 0:ng], ng, magic_t, "rsq")
                dist = grpp.tile([P, GS], F32, tag="dist")
                nc.vector.tensor_mul(dist[:, 0:ng], d2g[:, 0:ng], ry[:].bitcast(F32))

                # envelope: env = p(t)^2, t = min(d2/49, 1)
                tgeo = grpp.tile([P, GS], F32, tag="tgeo")
                nc.vector.tensor_scalar(
                    out=tgeo[:, 0:ng], in0=d2g[:, 0:ng], scalar1=1.0 / 49.0, scalar2=1.0,
                    op0=OP.mult, op1=OP.min,
                )
                envr = grpp.tile([P, GS], F32, tag="envr")
                nc.vector.tensor_scalar(
                    out=envr[:, 0:ng], in0=tgeo[:, 0:ng], scalar1=ENV_A[6], scalar2=None,
                    op0=OP.mult,
                )
                for k in range(5, 0, -1):
                    nc.vector.scalar_tensor_tensor(
                        out=envr[:, 0:ng], in0=envr[:, 0:ng], scalar=ENV_A[k],
                        in1=tgeo[:, 0:ng], op0=OP.add, op1=OP.mult,
                    )
                env = grpp.tile([P, GS], F32, tag="env")
                nc.vector.tensor_scalar(
                    out=env[:, 0:ng], in0=envr[:, 0:ng], scalar1=ENV_A[0], scalar2=None,
                    op0=OP.add,
                )
                nc.vector.tensor_mul(env[:, 0:ng], env[:, 0:ng], env[:, 0:ng])

                # rbf then demb = rbf * env (one Square + one Exp per group)
                rb = grpp.tile([P, GS, NB], F32, tag="rb")
                nc.vector.tensor_tensor(
                    out=rb[:, 0:ng, :],
                    in0=offs_sb[:].unsqueeze(1).to_broadcast([P, ng, NB]),
                    in1=dist[:, 0:ng].unsqueeze(2).to_broadcast([P, ng, NB]),
                    op=OP.subtract,
                )
                nc.scalar.activation(rb[:, 0:ng, :], rb[:, 0:ng, :], AF.Square, scale=sqc)
                nc.scalar.activation(rb[:, 0:ng, :], rb[:, 0:ng, :], AF.Exp, scale=-1.0)
                demb = grpp.tile([P, GS, NB], BF16, tag="demb")
                nc.vector.tensor_tensor(
                    out=demb[:, 0:ng, :], in0=rb[:, 0:ng, :],
                    in1=env[:, 0:ng].unsqueeze(2).to_broadcast([P, ng, NB]),
                    op=OP.mult,
                )

                # ======== Phase B: TP + LN + dfilter + MLP (silu table) ========
                for i, b in enumerate(blocks):
                    cls = b // NBLK_CLS
                    ps, pd = (cls >> 1) & 1, cls & 1
                    e0 = b * BLK
                    sl = slice(e0, e0 + BLK)
                    xg = xg_tiles.pop(b)

                    x1 = xg[:, 0:SUB, ps * XR : ps * XR + 120]
                    x2 = xg[:, SUB : 2 * SUB, pd * XR : pd * XR + 120]

                    psmix = ps_mm.tile([P, SUB, NS], F32, tag="psmix")
                    sumv = geop.tile([P, SUB], F32, tag="sumv")
                    sumsq = geop.tile([P, SUB], F32, tag="sumsq")

                    # ---- pass 1: tensor product per sub-tile ----
                    for s in range(SUB):
                        ptb = pfp.tile([P, KPAD], BF16, tag="ptb")
                        if n_ptb < 2:
                            nc.vector.memset(ptb[:, KTP:KPAD], 0.0)
                            n_ptb += 1
                        a1 = x1[:, s, 0:L0]
                        a2 = x2[:, s, 0:L0]
                        nc.vector.tensor_tensor(
                            out=ptb[:, 0:1024].rearrange("p (u v) -> p u v", v=L0),
                            in0=a1.unsqueeze(2).to_broadcast([P, L0, L0]),
                            in1=a2.unsqueeze(1).to_broadcast([P, L0, L0]),
                            op=OP.mult,
                        )
                        # 1o path: m-major product tile, reduce via 2 contiguous adds
                        b1 = x1[:, s, 32:80].rearrange("p (u m) -> p m u", m=3)
                        b2 = x2[:, s, 32:80].rearrange("p (v m) -> p m v", m=3)
                        pb = workp.tile([P, 3, L1, L1], BF16, tag="pb")
                        nc.vector.tensor_tensor(
                            out=pb[:],
                            in0=b1.unsqueeze(3).to_broadcast([P, 3, L1, L1]),
                            in1=b2.unsqueeze(2).to_broadcast([P, 3, L1, L1]),
                            op=OP.mult,
                        )
                        pbf = pb[:].rearrange("p m u v -> p m (u v)")
                        with nc.allow_low_precision(reason="3-term bf16 add"):
                            t01 = workp.tile([P, L1 * L1], BF16, tag="t01")
                            nc.vector.tensor_add(t01[:], pbf[:, 0, :], pbf[:, 1, :])
                            nc.vector.tensor_add(ptb[:, 1024:1280], t01[:], pbf[:, 2, :])
                        # 2e path
                        c1 = x1[:, s, 80:120].rearrange("p (u m) -> p m u", m=5)
                        c2 = x2[:, s, 80:120].rearrange("p (v m) -> p m v", m=5)
                        pc = workp.tile([P, 5, L2, L2], BF16, tag="pc")
                        nc.vector.tensor_tensor(
                            out=pc[:],
                            in0=c1.unsqueeze(3).to_broadcast([P, 5, L2, L2]),
                            in1=c2.unsqueeze(2).to_broadcast([P, 5, L2, L2]),
                            op=OP.mult,
                        )
                        pcf = pc[:].rearrange("p m u v -> p m (u v)")
                        with nc.allow_low_precision(reason="5-term bf16 add"):
                            u01 = workp.tile([P, L2 * L2], BF16, tag="u01")
                            nc.vector.tensor_add(u01[:], pcf[:, 0, :], pcf[:, 1, :])
                            u23 = workp.tile([P, L2 * L2], BF16, tag="u23")
                            nc.vector.tensor_add(u23[:], pcf[:, 2, :], pcf[:, 3, :])
                            nc.vector.tensor_add(u01[:], u01[:], u23[:])
                            nc.vector.tensor_add(ptb[:, 1280:1344], u01[:], pcf[:, 4, :])

                        # transposes: chunks 0-7 into one full PSUM bank (scalar
                        # copy), chunks 8-10 into a half bank (vector copy)
                        ptp8 = ps_t8.tile([P, 8, P], BF16, tag="ptp8")
                        for c in range(8):
                            nc.tensor.transpose(
                                ptp8[:, c, :], ptb[:, c * P : (c + 1) * P], identb[:]
                            )
                        pts8 = trsbp.tile([P, 8, P], BF16, tag="pts8")
                        nc.scalar.copy(pts8[:], ptp8[:])
                        ptp4 = ps_t8.tile([P, 8, P], BF16, tag="ptp8")
                        for j, c in enumerate((8, 9, 10)):
                            nc.tensor.transpose(
                                ptp4[:, j, :], ptb[:, c * P : (c + 1) * P], identb[:]
                            )
                        pts4 = trsbp.tile([P, 4, P], BF16, tag="pts4")
                        nc.vector.tensor_copy(pts4[:, 0:3, :], ptp4[:, 0:3, :])
                        for c in range(8):
                            nc.tensor.matmul(
                                psmix[:, s, :], lhsT=pts8[:, c, :], rhs=w_sb[:, c, :],
                                start=(c == 0), stop=False,
                            )
                        for j, c in enumerate((8, 9, 10)):
                            nc.tensor.matmul(
                                psmix[:, s, :], lhsT=pts4[:, j, :], rhs=w_sb[:, c, :],
                                start=False, stop=(c == NCHUNK - 1),
                            )

                    # ---- block-level LN stats: mean + sumsq reduces ----
                    nc.vector.reduce_sum(
                        out=sumv[:], in_=psmix[:], axis=mybir.AxisListType.X,
                    )
                    sq = workp.tile([P, SUB, NS], BF16, tag="sq")
                    nc.scalar.activation(sq[:], psmix[:], AF.Square)
                    with nc.allow_low_precision(reason="bf16 sumsq reduce"):
                        nc.vector.reduce_sum(
                            out=sumsq[:], in_=sq[:], axis=mybir.AxisListType.X,
                        )
                    muv = geop.tile([P, SUB], F32, tag="muv")
                    nc.vector.tensor_scalar(
                        out=muv[:], in0=sumv[:], scalar1=1.0 / NS, scalar2=None, op0=OP.mult,
                    )
                    varv = geop.tile([P, SUB], F32, tag="varv")
                    nc.vector.tensor_mul(varv[:], muv[:], muv[:])
                    nc.vector.scalar_tensor_tensor(
                        out=varv[:], in0=sumsq[:], scalar=1.0 / NS, in1=varv[:],
                        op0=OP.mult, op1=OP.subtract,
                    )
                    nc.vector.tensor_scalar(
                        out=varv[:], in0=varv[:], scalar1=1e-5, scalar2=None, op0=OP.add,
                    )
                    ryl = _newton_rsqrt(nc, geop, varv[:], SUB, magic_t, "lnr")
                    rstd = ryl[:].bitcast(F32)
                    tb = geop.tile([P, SUB], F32, tag="tb")
                    nc.vector.tensor_mul(tb[:], muv[:], rstd)
                    nc.vector.tensor_scalar(
                        out=tb[:], in0=tb[:], scalar1=-1.0, scalar2=None, op0=OP.mult,
                    )

                    acc = accp.tile([P, SUB], F32, tag="acc")

                    # batched dT transposes: two sub-tiles of demb per transpose
                    dTs = []
                    for h in range(2):
                        dT_ps = ps_t8.tile([P, 8, P], BF16, tag="ptp8")
                        nc.tensor.transpose(
                            dT_ps[:, 0, :],
                            demb[:, (i * SUB + 2 * h) : (i * SUB + 2 * h + 2), :]
                            .rearrange("p s k -> p (s k)"),
                            identb[:],
                        )
                        dT = trsbp.tile([P, P], BF16, tag=f"dT{h}")
                        nc.scalar.copy(dT[:], dT_ps[:, 0, :])
                        dTs.append(dT)

                    # ---- pass 2: LN apply + dfilter + final MLP ----
                    for s in range(SUB):
                        ynorm = workp.tile([P, NS], BF16, tag="ynorm")
                        nc.scalar.activation(
                            ynorm[:], psmix[:, s, :], AF.Identity,
                            bias=tb[:, s : s + 1], scale=rstd[:, s : s + 1],
                        )

                        h0 = (s % 2) * NB
                        dT = dTs[s // 2][h0 : h0 + NB, :]
                        rhs1 = dfw1_sb[h0 : h0 + NB, :]
                        ph = ps_hd.tile([P, 128], F32, tag="ph")
                        if zero_bias:
                            nc.tensor.matmul(ph[:], lhsT=dT, rhs=rhs1, start=True, stop=True)
                        else:
                            nc.tensor.matmul(ph[:], lhsT=dT, rhs=rhs1, start=True, stop=False)
                            nc.tensor.matmul(ph[:], lhsT=ones_row[:], rhs=dfb1_sb[:], start=False, stop=True)
                        sact = workp.tile([P, 128], BF16, tag="sact")
                        nc.scalar.activation(sact[:], ph[:], AF.Silu)
                        sT_ps = ps_t8.tile([P, 8, P], BF16, tag="ptp8")
                        nc.tensor.transpose(sT_ps[:, 0, :], sact[:], identb[:])
                        sT = trsbp.tile([P, P], BF16, tag="sT")
                        nc.vector.tensor_copy(sT[:], sT_ps[:, 0, :])
                        pdf = ps_hd.tile([P, 256], F32, tag="pdf")
                        if zero_bias:
                            nc.tensor.matmul(pdf[:], lhsT=sT[:], rhs=dfw2gb_sb[:], start=True, stop=True)
                        else:
                            nc.tensor.matmul(pdf[:], lhsT=sT[:], rhs=dfw2gb_sb[:], start=True, stop=False)
                            nc.tensor.matmul(pdf[:], lhsT=ones_row[:], rhs=dfb2gb_sb[:], start=False, stop=True)

                        rg = workp.tile([P, 128], BF16, tag="rg")
                        nc.vector.tensor_mul(rg[:], ynorm[:], pdf[:, 0:128])
                        nc.vector.tensor_add(rg[:], rg[:], pdf[:, 128:256])

                        rT_ps = ps_t8.tile([P, 8, P], BF16, tag="ptp8")
                        nc.tensor.transpose(rT_ps[:, 0, :], rg[:], identb[:])
                        rT = trsbp.tile([P, P], BF16, tag="rT")
                        nc.scalar.copy(rT[:], rT_ps[:, 0, :])
                        pg2 = ps_g.tile([P, 512], F32, tag="pg")
                        if zero_bias:
                            nc.tensor.matmul(pg2[:], lhsT=rT[:], rhs=mlpw1_sb[:], start=True, stop=True)
                        else:
                            nc.tensor.matmul(pg2[:], lhsT=rT[:], rhs=mlpw1_sb[:], start=True, stop=False)
                            nc.tensor.matmul(pg2[:], lhsT=ones_row[:], rhs=mlpb1_sb[:], start=False, stop=True)
                        gact = workp.tile([P, 512], BF16, tag="gact")
                        nc.scalar.activation(gact[:], pg2[:], AF.Silu)
                        scr = workp.tile([P, 512], BF16, tag="scr")
                        nc.vector.scalar_tensor_tensor(
                            out=scr[:], in0=gact[:], scalar=1.0, in1=w2rep_sb[:],
                            op0=OP.mult, op1=OP.mult,
                            accum_out=acc[:, s : s + 1],
                        )

                    if not zero_bias:
                        nc.vector.tensor_scalar(
                            out=acc[:], in0=acc[:], scalar1=b2_sb[:, 0:1], scalar2=None,
                            op0=OP.add,
                        )
                    nc.sync.dma_start(out=out[sl].rearrange("(s p) -> p s", p=P), in_=acc[:])

    nc.compile()
    return nc


def _get_compiled(zero_bias: bool = True):
    if zero_bias not in _compiled:
        _compiled[zero_bias] = _build(NBLK, zero_bias)
    return _compiled[zero_bias]


def _wrap16(idx_block):
    """int array [512] -> dma_gather wrapped int16 layout [128, 32]
    (index j at [j%16, j//16], replicated across the 8 gpsimd cores)."""
    w = idx_block.astype(np.int16).reshape(-1, 16).T  # [16, n/16]
    return np.tile(w, (8, 1))


def _prep(inputs):
    nodes = np.asarray(inputs["nodes"], np.float32)
    edge_index = np.asarray(inputs["edge_index"]).astype(np.int64)
    graph_batch = np.asarray(inputs["graph_batch"]).astype(np.int64)
    cell = np.asarray(inputs["cell"], np.float32).reshape(32, 9)
    edge_shift = np.asarray(inputs["edge_shift"], np.float32)
    pos = np.asarray(inputs["pos"], np.float32)

    # bf16 pair-row node table with fp32 pos bit-packed at units 120:126
    row_u16 = np.zeros((N_NODES, XR), np.uint16)
    row_u16[:, 0:NODE_DIM] = nodes.astype(ml_dtypes.bfloat16).view(np.uint16)
    row_u16[:, 120:126] = pos.view(np.uint16).reshape(N_NODES, 6)
    nodes_pair = row_u16.reshape(NPAIR, 2 * XR).view(ml_dtypes.bfloat16)

    alpha = 1.0 / np.sqrt(float(L0 * L0 + L1 * L1 + L2 * L2))
    w0 = np.asarray(inputs["W0"], np.float32).reshape(L0 * L0, NS) * alpha
    w1 = np.asarray(inputs["W1"], np.float32).reshape(L1 * L1, NS) * (alpha / np.sqrt(3.0))
    w2 = np.asarray(inputs["W2"], np.float32).reshape(L2 * L2, NS) * (alpha / np.sqrt(5.0))
    wflat = np.zeros((KPAD, NS), np.float32)
    wflat[0:1024] = w0
    wflat[1024:1280] = w1
    wflat[1280:1344] = w2

    ln_g = np.asarray(inputs["ln_g"], np.float32)
    ln_b = np.asarray(inputs["ln_b"], np.float32)
    df_w2 = np.asarray(inputs["df_w2"], np.float32)
    df_b2 = np.asarray(inputs["df_b2"], np.float32)
    dfw2gb = np.concatenate([df_w2 * ln_g[None, :], df_w2 * ln_b[None, :]], axis=1)
    dfb2gb = np.concatenate([df_b2 * ln_g, df_b2 * ln_b])[None, :]

    zero_bias = (
        not np.any(np.asarray(inputs["df_b1"]))
        and not np.any(np.asarray(inputs["df_b2"]))
        and not np.any(np.asarray(inputs["mlp_b1"]))
        and not np.any(np.asarray(inputs["mlp_b2"]))
        and not np.any(ln_b)
    )

    bf = lambda a: np.ascontiguousarray(a).astype(ml_dtypes.bfloat16)

    common = {
        "nodes_pair": nodes_pair,
        "wflat": bf(wflat),
        "dfw1": bf(np.asarray(inputs["df_w1"], np.float32)),
        "dfb1": bf(np.asarray(inputs["df_b1"], np.float32)[None, :]),
        "dfw2gb": bf(dfw2gb),
        "dfb2gb": bf(dfb2gb),
        "mlpw1": bf(np.asarray(inputs["mlp_w1"], np.float32)),
        "mlpb1": bf(np.asarray(inputs["mlp_b1"], np.float32)[None, :]),
        "w2row": bf(np.asarray(inputs["mlp_w2"], np.float32).T),
        "b2sc": np.asarray(inputs["mlp_b2"], np.float32).reshape(1, 1),
        "offs": np.linspace(0.0, CUTOFF, NB, dtype=np.float32)[None, :],
    }

    in_maps = []
    outmaps = []
    for c in range(NCORES):
        lo, hi = c * E_CORE, (c + 1) * E_CORE
        src = edge_index[0, lo:hi]
        dst = edge_index[1, lo:hi]
        esh = edge_shift[lo:hi]
        key = ((src & 1) << 1) | (dst & 1)

        srcp = np.zeros(E_PAD, np.int64)
        dstp = np.zeros(E_PAD, np.int64)
        geo = np.zeros((E_PAD, 12), np.float32)
        outmap = np.full(E_PAD, -1, np.int64)
        for cls in range(4):
            idxs = np.nonzero(key == cls)[0]
            n = len(idxs)
            assert n <= ECLS, f"class {cls} overflow: {n} > {ECLS}"
            base = cls * ECLS
            srcp[base : base + n] = src[idxs]
            dstp[base : base + n] = dst[idxs]
            geo[base : base + n, 0:3] = esh[idxs]
            geo[base : base + n, 3:12] = cell[graph_batch[src[idxs]]]
            outmap[base : base + n] = idxs

        xw = np.zeros((NBLK, P, 64), np.int16)
        for b in range(NBLK):
            sb = srcp[b * BLK : (b + 1) * BLK]
            db = dstp[b * BLK : (b + 1) * BLK]
            xw[b, :, 0:32] = _wrap16(sb >> 1)
            xw[b, :, 32:64] = _wrap16(db >> 1)

        m = dict(common)
        m["xw16"] = xw
        m["geo12"] = geo
        in_maps.append(m)
        outmaps.append(outmap)
    return in_maps, outmaps, zero_bias


def _gather_out(res, outmaps, b2_host=0.0):
    full = np.empty((N_EDGES,), np.float32)
    for c in range(NCORES):
        dev = np.asarray(res.results[c]["out"])
        outmap = outmaps[c]
        valid = outmap >= 0
        full[c * E_CORE + outmap[valid]] = dev[valid]
    return full.reshape(N_EDGES, 1)


def kernel(**inputs) -> np.ndarray:
    in_maps, outmaps, zero_bias = _prep(inputs)
    nc = _get_compiled(zero_bias)
    res = run_bass_kernel_spmd(nc, in_maps, core_ids=list(range(NCORES)))
    return _gather_out(res, outmaps)
